# revision 3
# baseline (speedup 1.0000x reference)
"""Multi-head causal attention (B=2, S=2048, D=1024, H=16) on 8 TRN2 NeuronCores.

Sharding: batch*head parallel. Core c handles batch b = c//4 and the 4
heads h in [4*(c%4), 4*(c%4)+4). Each core computes its heads' Q/K/V
projections (column-parallel), causal softmax attention, and its partial
row-parallel output projection; the host sums the 4 partial outputs per
batch (the AllReduce of row-parallel tensor parallelism).

v2 design vs v1 (311us baseline):
  - all matmul operands bf16 (host-cast): projection-phase DMA halves
    (24MB f32 -> 12MB bf16 of x per core), LDWEIGHTS cheaper.
  - xv host-shuffled chunk-major so every DMA line is 2KB contiguous.
  - psS/psO double-buffered: the attnV accumulation of q-block j+1 no
    longer waits for the softmax-normalize of block j (the 2.75us PE
    stall per block boundary that kept re-triggering the PE HAM
    half-clock throttle).
  - normalize chain shrunk to reciprocal([1,1024] on PSUM sums row) +
    gpsimd partition_broadcast + 2 DVE muls; PSUM->SBUF y copies moved
    to the idle GpSimd engine so Vector stays off the critical path.
  - causally-dead column windows of diagonal blocks are skipped in the
    score matmuls, exp, and attnV (cols < 128r of a diag-r tile are
    fully masked -> contribute exactly 0).
  - exp table preloaded via a dummy activation at t=0 (hides the ~2.7us
    ACT table-set load).
Softmax skips the max-subtraction: scores ~ N(0,1), so exp never
overflows fp32, and exp(-1e9/8) underflows to exactly 0 like the
reference's masked_fill(-1e9).
"""

import numpy as np

D_MODEL = 1024
N_HEADS = 16
D_K = 64
B, S = 2, 2048
N_CORES = 8
HPC = 4            # heads per core
KT = S // 128      # 16 k-tiles
QT = S // 512      # 4 q-tiles
ET = D_MODEL // 128  # 8 e-tiles (contraction tiles for projections)

WARM_MMS = 56
DIAG_TRIM = True   # skip fully-masked col windows of diagonal tiles
NEW_NORM = True    # recip+partition_broadcast normalize (vs v1-style)
SPREAD_OUTPROJ = True
V_SPLIT = False    # project V chunks 8-15 inside the pr0 attention pass

_PROG_CACHE = {}


def _build_program():
    import concourse.bacc as bacc_mod
    import concourse.mybir as mybir
    import concourse.tile as tile

    f32 = mybir.dt.float32
    bf16 = mybir.dt.bfloat16
    Exp = mybir.ActivationFunctionType.Exp

    nc = bacc_mod.Bacc(
        "TRN2", target_bir_lowering=False, debug=False, num_devices=N_CORES
    )

    xq = nc.dram_tensor("xq", [D_MODEL, S], bf16, kind="ExternalInput").ap()
    xk = nc.dram_tensor("xk", [D_MODEL, S], bf16, kind="ExternalInput").ap()
    xvc = nc.dram_tensor("xvc", [128, KT * ET * 128], bf16, kind="ExternalInput").ap()
    wq = nc.dram_tensor("wq", [128, ET * 256], bf16, kind="ExternalInput").ap()
    wk = nc.dram_tensor("wk", [128, ET * 256], bf16, kind="ExternalInput").ap()
    wv = nc.dram_tensor("wv", [128, ET * 256], bf16, kind="ExternalInput").ap()
    wo = nc.dram_tensor("wo", [256, D_MODEL], bf16, kind="ExternalInput").ap()
    maskt = nc.dram_tensor("maskt", [128, 2048], bf16, kind="ExternalInput").ap()
    idbf = nc.dram_tensor("idbf", [128, 196], bf16, kind="ExternalInput").ap()
    y = nc.dram_tensor("y", [S, D_MODEL], bf16, kind="ExternalOutput").ap()

    with (
        tile.TileContext(nc) as tc,
        nc.allow_low_precision("bf16 attention"),
        tc.tile_pool(name="persist", bufs=1) as pp,
    ):
        # ---- persistent SBUF tiles ----
        def persist(shape, dtype, name):
            return pp.tile(shape, dtype, name=name, tag=name)

        wq_sb = persist([128, ET * 256], bf16, "wq_sb")
        wk_sb = persist([128, ET * 256], bf16, "wk_sb")
        wv_sb = persist([128, ET * 256], bf16, "wv_sb")
        wo_sb = [persist([128, D_MODEL], bf16, f"wo_sb{p}") for p in range(2)]
        maskt_sb = persist([128, 2048], bf16, "maskt_sb")
        idbf_sb = persist([128, 196], bf16, "idbf_sb")
        qt_sb = [persist([128, S], bf16, f"qt_sb{p}") for p in range(2)]
        kt_sb = [persist([128, S], bf16, f"kt_sb{p}") for p in range(2)]
        v_sb = [persist([128, 260], bf16, f"v_sb{i}") for i in range(KT)]
        outt_sb = [persist([128, S], bf16, f"outt_sb{p}") for p in range(2)]
        exp_warm = persist([128, 1], f32, "exp_warm")

        nc.sync.dma_start(out=idbf_sb[:], in_=idbf[:])
        maskt_dram = maskt
        # preload the Exp table-set during the PE warm-up (~2.7us ACT load)
        nc.scalar.activation(exp_warm[:], idbf_sb[:, 0:1], Exp, scale=0.125)
        # ones columns of v_sb (col 64 of each 65-wide head slot) never
        # change: write them once, early, on gpsimd.
        for i in range(KT):
            nc.gpsimd.tensor_copy(
                v_sb[i][:].rearrange("p (h c) -> p h c", c=65)[:, :, 64:65],
                idbf_sb[:, 192:196].rearrange("p (h c) -> p h c", c=1),
            )
        nc.sync.dma_start(out=wq_sb[:], in_=wq[:])

        # ---- PE warm-up ----
        # The PE HAM clock gate starts at K=4/8 half-clock and returns to
        # full clock only after ~3.4us of gapless PE activity. Dense dummy
        # matmuls (results never read) force the transition while the
        # first x DMAs are in flight.
        with tc.tile_pool(name="psW", bufs=1, space="PSUM") as psW:
            wt = psW.tile([128, 128], f32, name="warm_start", tag="warm")
            for w in range(WARM_MMS):
                nc.tensor.matmul(
                    wt[:],
                    idbf_sb[:, 0:128],
                    idbf_sb[:, 64:192],
                    start=True,
                    stop=True,
                )

        # ---- phase B: projections ----
        # Q^T/K^T accumulate over all 8 e-tiles into [128, 2048] PSUM.
        # xv chunk DMAs (contiguous thanks to the host shuffle) are
        # interleaved so the V projection starts with its data resident.
        xvk_ctx = tc.tile_pool(name="xvk", bufs=16)
        xvkp = xvk_ctx.__enter__()
        with tc.tile_pool(name="xe", bufs=5) as xep:
            vdma_tiles = []

            def emit_v_dma():
                i = len(vdma_tiles)
                xvk = xvkp.tile([128, ET * 128], bf16, name=f"xvk_{i}", tag="xvk")
                nc.sync.dma_start(
                    out=xvk[:], in_=xvc[:, i * ET * 128 : (i + 1) * ET * 128]
                )
                vdma_tiles.append(xvk)

            psA_ctx = tc.tile_pool(name="psA", bufs=1, space="PSUM")
            psA = psA_ctx.__enter__()
            for ti, (x_dram, w_tile, dst) in enumerate(
                ((xq, wq_sb, qt_sb), (xk, wk_sb, kt_sb))
            ):
                ps = [
                    psA.tile(
                        [128, S], f32, name=f"ps_p{ti}_{m}", tag=f"proj{m}", bufs=1
                    )
                    for m in range(2)
                ]
                for e in range(ET):
                    xe = xep.tile([128, S], bf16, name=f"xe_{ti}_{e}", tag="xe")
                    nc.sync.dma_start(out=xe[:], in_=x_dram[e * 128 : (e + 1) * 128, :])
                    if ti == 0 and e == 1:
                        # prefetch mask + wk behind the first q x-tiles
                        nc.sync.dma_start(out=maskt_sb[:], in_=maskt_dram[:])
                        nc.sync.dma_start(out=wk_sb[:], in_=wk[:])
                    if ti == 1 and e == 0:
                        nc.sync.dma_start(out=wv_sb[:], in_=wv[:])
                        for p in range(2):
                            nc.sync.dma_start(
                                out=wo_sb[p][:], in_=wo[p * 128 : (p + 1) * 128, :]
                            )
                    if ti == 1 or e >= 1:
                        emit_v_dma()
                    for m in range(2):
                        lhsT = w_tile[:, e * 256 + m * 128 : e * 256 + (m + 1) * 128]
                        for n in range(QT):
                            nc.tensor.matmul(
                                ps[m][:, n * 512 : (n + 1) * 512],
                                lhsT,
                                xe[:, n * 512 : (n + 1) * 512],
                                start=(e == 0),
                                stop=(e == ET - 1),
                            )
                # PSUM -> SBUF casts: q on vector, k on scalar (parallel;
                # gpsimd cannot access PSUM on TRN2)
                for m in range(2):
                    if ti == 0:
                        nc.vector.tensor_copy(dst[m][:], ps[m][:])
                    else:
                        nc.scalar.activation(
                            dst[m][:], ps[m][:],
                            mybir.ActivationFunctionType.Copy,
                        )

            psA_ctx.__exit__(None, None, None)
            psV_ctx = tc.tile_pool(name="psV", bufs=2, space="PSUM")
            psV = psV_ctx.__enter__()
            # V projection: dense PE burst, v_sb tiles ready incrementally
            # (with V_SPLIT, chunks 8-15 are projected later, inside the
            # Act-bound pr0 attention blocks that first consume them)
            for i in range(KT // 2 if V_SPLIT else KT):
                if i >= len(vdma_tiles) - 2 and len(vdma_tiles) < KT:
                    emit_v_dma()
                psv = psV.tile([128, 256], f32, name=f"psv_{i}", tag="v")
                xvk = vdma_tiles[i]
                for e in range(ET):
                    nc.tensor.matmul(
                        psv[:],
                        xvk[:, e * 128 : (e + 1) * 128],
                        wv_sb[:, e * 256 : (e + 1) * 256],
                        start=(e == 0),
                        stop=(e == ET - 1),
                    )
                nc.vector.tensor_copy(
                    v_sb[i][:].rearrange("p (h c) -> p h c", c=65)[:, :, 0:64],
                    psv[:].rearrange("p (h d) -> p h d", d=64),
                )
            while len(vdma_tiles) < KT:
                emit_v_dma()
            psV_ctx.__exit__(None, None, None)

        # ---- phase C+D: attention with interleaved output projection ----
        # One head-pair per pass (pr = 0, 1), q-blocks ascending. Per
        # (pr, j): score tiles are [128, 1024] head-pair PSUM tiles; ONE
        # exp per round at FD~1024. attnV accumulates into a [65, 1024]
        # pair tile (row 64 = sum of exp via the ones column of v_sb).
        # Normalization of the previous block is emitted at the top of
        # the next block (psO is double-buffered so nothing on the PE
        # waits for it); output projection of block j-1 is spread over
        # i = 2..5 of block j during the pr=1 pass. Diag-r tiles skip
        # their fully-masked first 128*r columns everywhere.
        with (
            tc.tile_pool(name="psS", bufs=2, space="PSUM") as psS,
            tc.tile_pool(name="psO", bufs=2, space="PSUM") as psO,
            tc.tile_pool(name="et", bufs=6) as etp,
            tc.tile_pool(name="bcsb", bufs=3) as bcp,
            tc.tile_pool(name="rcsb", bufs=3) as rcp,
            tc.tile_pool(name="ysb", bufs=4) as ysbp,
        ):
            def emit_outproj_mtile(m):
                psy = psS.tile([128, 1024], f32, name=f"psy_{m}", tag="s")
                for p in range(2):
                    for n in range(2):
                        nc.tensor.matmul(
                            psy[:, n * 512 : (n + 1) * 512],
                            outt_sb[p][:, m * 128 : (m + 1) * 128],
                            wo_sb[p][:, n * 512 : (n + 1) * 512],
                            start=(p == 0),
                            stop=(p == 1),
                        )
                y_sb = ysbp.tile([128, 1024], bf16, name=f"y_sb_{m}", tag="ysb")
                nc.vector.tensor_copy(y_sb[:], psy[:])
                nc.sync.dma_start(out=y[m * 128 : (m + 1) * 128, :], in_=y_sb[:])

            def emit_normalize(pr, jj, ps_out_prev):
                qsj = slice(jj * 512, (jj + 1) * 512)
                if NEW_NORM:
                    # custom-DVE ops silently misread PSUM: stage the sums
                    # row into SBUF (partition 0) before the reciprocal.
                    # Split per head so the first multiply starts after a
                    # half-size copy+recip+broadcast chain.
                    bc_sb = bcp.tile(
                        [64, 1024], f32, name=f"bc_sb_{pr}_{jj}", tag="bc"
                    )
                    for hh in range(2):
                        hs = slice(512 * hh, 512 * (hh + 1))
                        ssb = rcp.tile(
                            [1, 512], f32, name=f"ssb_{pr}_{jj}_{hh}", tag="rc32"
                        )
                        nc.vector.tensor_copy(ssb[:], ps_out_prev[64:65, hs])
                        rc32 = rcp.tile(
                            [1, 512], f32, name=f"rc32_{pr}_{jj}_{hh}", tag="rc32"
                        )
                        nc.vector.reciprocal_approx_fast(out=rc32[:], in_=ssb[:])
                        nc.gpsimd.partition_broadcast(bc_sb[:, hs], rc32[:])
                else:
                    ssb = rcp.tile([33, 512], f32, name=f"ssb_{pr}_{jj}", tag="rc32")
                    for hh in range(2):
                        nc.vector.tensor_copy(
                            ssb[32 * hh : 32 * hh + 1, :],
                            ps_out_prev[64:65, 512 * hh : 512 * (hh + 1)],
                        )
                    rc32 = rcp.tile([33, 512], f32, name=f"rc_{pr}_{jj}", tag="rc32")
                    nc.vector.reciprocal_approx_fast(out=rc32[:], in_=ssb[:])
                    rcb = rcp.tile([33, 512], bf16, name=f"rcb_{pr}_{jj}", tag="rc32")
                    nc.vector.tensor_copy(rcb[:], rc32[:])
                    bcps = psS.tile([128, 1024], f32, name=f"bcp_{pr}_{jj}", tag="s")
                    for hh in range(2):
                        nc.tensor.matmul(
                            bcps[0:64, 512 * hh : 512 * (hh + 1)],
                            idbf_sb[32 * hh : 32 * hh + 1, 128:192],
                            rcb[32 * hh : 32 * hh + 1, :],
                            start=True,
                            stop=True,
                            tile_position=(32 * hh, 0),
                        )
                    bc_sb = bcp.tile(
                        [64, 1024], f32, name=f"bc_sb_{pr}_{jj}", tag="bc"
                    )
                    nc.vector.tensor_copy(bc_sb[:], bcps[0:64, :])
                for hh in range(2):
                    nc.vector.tensor_mul(
                        outt_sb[pr][64 * hh : 64 * hh + 64, qsj],
                        ps_out_prev[0:64, 512 * hh : 512 * (hh + 1)],
                        bc_sb[:, 512 * hh : 512 * (hh + 1)],
                    )

            pending_norm = None  # (pr, j, ps_out) awaiting lazy normalize
            pending_out = []     # outproj m-tiles awaiting emission (pr=1)
            carry = None         # final attnV of the previous block, emitted
                                 # after the next block's first scores+exp

            def emit_carry():
                c_et, c_i, c_w0, c_psout, c_pr, c_ni = carry
                for hh in range(2):
                    nc.tensor.matmul(
                        c_psout[:, 512 * hh + c_w0 : 512 * (hh + 1)],
                        v_sb[c_ni - 1][
                            :, (2 * c_pr + hh) * 65 : (2 * c_pr + hh + 1) * 65
                        ],
                        c_et[:, 512 * hh + c_w0 : 512 * (hh + 1)],
                        start=(c_ni - 1 == 0),
                        stop=True,
                    )

            for pr in range(2):
                for j in range(QT):
                    n_i = 4 * j + 4
                    ps_out = psO.tile(
                        [65, 1024], f32, name=f"ps_out_{pr}_{j}", tag="o"
                    )
                    prev_et = None
                    prev_i = -1
                    prev_w0 = 0
                    for i in range(n_i):
                        diag = i >= 4 * j
                        r = i - 4 * j
                        w0 = 128 * r if (diag and DIAG_TRIM) else 0
                        pss = psS.tile(
                            [128, 1024], f32, name=f"ps_s{pr}_{j}_{i}", tag="s"
                        )
                        for hh in range(2):
                            hp = slice(64 * hh, 64 * hh + 64)
                            nc.tensor.matmul(
                                pss[:, 512 * hh + w0 : 512 * (hh + 1)],
                                kt_sb[pr][hp, i * 128 : (i + 1) * 128],
                                qt_sb[pr][hp, j * 512 + w0 : (j + 1) * 512],
                                start=True,
                                stop=not diag,
                            )
                        if diag:
                            # band-only causal mask add (the straddle is
                            # 128 cols; beyond it the mask is zero)
                            bw = min(128 * (r + 1), 512)
                            for hh in range(2):
                                nc.tensor.matmul(
                                    pss[:, 512 * hh + w0 : 512 * hh + bw],
                                    idbf_sb[:, 0:128],
                                    maskt_sb[:, r * 512 + w0 : r * 512 + bw],
                                    start=False,
                                    stop=True,
                                )
                        et = etp.tile(
                            [128, 1024], bf16, name=f"et{pr}_{j}_{i}", tag="et"
                        )
                        if w0:
                            nc.scalar.activation(
                                et[:].rearrange("p (h q) -> p h q", h=2)[
                                    :, :, w0:512
                                ],
                                pss[:].rearrange("p (h q) -> p h q", h=2)[
                                    :, :, w0:512
                                ],
                                Exp,
                                scale=0.125,
                            )
                        else:
                            nc.scalar.activation(et[:], pss[:], Exp, scale=0.125)
                        if i == 0:
                            # previous block's final attnV, pipelined behind
                            # this block's first scores so its exp isn't
                            # delayed by the accumulation tail
                            if carry is not None:
                                emit_carry()
                                pending_norm = (carry[4], None, carry[3])
                                carry = None
                        else:
                            for hh in range(2):
                                nc.tensor.matmul(
                                    ps_out[:, 512 * hh + prev_w0 : 512 * (hh + 1)],
                                    v_sb[prev_i][
                                        :, (2 * pr + hh) * 65 : (2 * pr + hh + 1) * 65
                                    ],
                                    prev_et[:, 512 * hh + prev_w0 : 512 * (hh + 1)],
                                    start=(prev_i == 0),
                                    stop=False,
                                )
                        if i == 1 and pending_norm is not None:
                            pn_pr, pn_j, pn_psout = pending_norm
                            pn_j2 = (j - 1) % QT if pn_j is None else pn_j
                            emit_normalize(pn_pr, pn_j2, pn_psout)
                            pending_norm = None
                            if pn_pr == 1:
                                pending_out = list(
                                    range(4 * pn_j2, 4 * pn_j2 + 4)
                                )
                        if V_SPLIT and pr == 0 and j >= 2 and i < 4:
                            c = 4 * j + i
                            if i % 2 == 0:
                                vps = psS.tile(
                                    [128, 512], f32, name=f"psv_{c}", tag="s"
                                )
                            vc = slice(256 * (i % 2), 256 * (i % 2) + 256)
                            xvk = vdma_tiles[c]
                            for e in range(ET):
                                nc.tensor.matmul(
                                    vps[:, vc],
                                    xvk[:, e * 128 : (e + 1) * 128],
                                    wv_sb[:, e * 256 : (e + 1) * 256],
                                    start=(e == 0),
                                    stop=(e == ET - 1),
                                )
                            nc.vector.tensor_copy(
                                v_sb[c][:].rearrange("p (h c) -> p h c", c=65)[
                                    :, :, 0:64
                                ],
                                vps[:, vc].rearrange("p (h d) -> p h d", d=64),
                            )
                        prev_et, prev_i, prev_w0 = et, i, w0
                        if pending_out and i >= 2 and i % 2 == 0:
                            emit_outproj_mtile(pending_out.pop(0))
                    while pending_out:
                        emit_outproj_mtile(pending_out.pop(0))
                    carry = (prev_et, prev_i, prev_w0, ps_out, pr, n_i)
            # tail: final attnV + normalize + project the last q-block.
            # Dummy matmuls keep the PE HAM at full clock through the
            # normalize chain so the final outproj isn't half-clocked.
            emit_carry()
            emit_normalize(1, QT - 1, carry[3])
            wt = psS.tile([128, 512], f32, name="warm_tail", tag="s")
            for _ in range(16):
                nc.tensor.matmul(
                    wt[:], idbf_sb[:, 0:128], maskt_sb[:, 0:512],
                    start=True, stop=True,
                )
            for m in range(4 * (QT - 1), 4 * QT):
                emit_outproj_mtile(m)

        xvk_ctx.__exit__(None, None, None)

    nc.compile()
    return nc


def _get_program():
    if "nc" not in _PROG_CACHE:
        _PROG_CACHE["nc"] = _build_program()
    return _PROG_CACHE["nc"]


def _host_prep(query, key, value, mask, w_q, w_k, w_v, w_o):
    import ml_dtypes

    bf = ml_dtypes.bfloat16
    query = np.asarray(query, dtype=np.float32)
    key = np.asarray(key, dtype=np.float32)
    value = np.asarray(value, dtype=np.float32)
    w_q = np.asarray(w_q, dtype=np.float32)
    w_k = np.asarray(w_k, dtype=np.float32)
    w_v = np.asarray(w_v, dtype=np.float32)
    w_o = np.asarray(w_o, dtype=np.float32)
    m = np.asarray(mask).reshape(S, S).astype(bool)

    # The kernel's block-skip structure assumes the standard causal mask.
    expected = np.triu(np.ones((S, S), dtype=bool), k=1)
    if not np.array_equal(m, expected):
        raise NotImplementedError("kernel specialized for causal (triu, k=1) mask")

    # 4 canonical diagonal-straddle mask tiles: pattern r covers k-tile
    # 4j+r vs q-tile j; masked where (128r + row) > col.
    maskt = np.zeros((128, 2048), dtype=np.float32)
    rows = np.arange(128)[:, None]
    cols = np.arange(512)[None, :]
    for r in range(4):
        maskt[:, r * 512 : (r + 1) * 512] = np.where(
            (128 * r + rows) > cols, np.float32(-1e9), np.float32(0.0)
        )
    maskt = maskt.astype(bf)
    idbf = np.zeros((128, 196), dtype=bf)
    idbf[:, 0:128] = np.eye(128, dtype=bf)
    idbf[:, 128:196] = bf(1.0)

    def wslab(w, rs):
        # [1024 in, 256 out] -> [128, 8*256] with e-tile t at cols 256t
        wt = np.ascontiguousarray(w[rs, :].T).astype(bf)  # [1024, 256]
        return np.ascontiguousarray(
            wt.reshape(ET, 128, 256).transpose(1, 0, 2).reshape(128, ET * 256)
        )

    xt = {}
    for b in range(B):
        xt[("q", b)] = np.ascontiguousarray(query[b].T).astype(bf)
        xt[("k", b)] = np.ascontiguousarray(key[b].T).astype(bf)
        # chunk-major xv: chunk i contiguous [128, 1024] (2KB lines)
        vt = np.ascontiguousarray(value[b].T).astype(bf)
        xt[("v", b)] = np.ascontiguousarray(
            vt.reshape(ET, 128, KT, 128)
            .transpose(1, 2, 0, 3)
            .reshape(128, KT * ET * 128)
        )

    in_maps = []
    for c in range(N_CORES):
        b = c // 4
        hb = (c % 4) * HPC
        rs = slice(hb * D_K, (hb + HPC) * D_K)
        in_maps.append(
            {
                "xq": xt[("q", b)],
                "xk": xt[("k", b)],
                "xvc": xt[("v", b)],
                "wq": wslab(w_q, rs),
                "wk": wslab(w_k, rs),
                "wv": wslab(w_v, rs),
                "wo": np.ascontiguousarray(w_o[:, rs].T).astype(bf),
                "maskt": maskt,
                "idbf": idbf,
            }
        )
    return in_maps


def kernel(query, key, value, mask, w_q, w_k, w_v, w_o):
    from concourse.bass_utils import run_bass_kernel_spmd

    in_maps = _host_prep(query, key, value, mask, w_q, w_k, w_v, w_o)
    nc = _get_program()
    res = run_bass_kernel_spmd(nc, in_maps, list(range(N_CORES)))
    out = np.zeros((B, S, D_MODEL), dtype=np.float32)
    for c in range(N_CORES):
        out[c // 4] += res.results[c]["y"].astype(np.float32)
    return out


# revision 4
# speedup vs baseline: 1.1388x; 1.1388x over previous
"""Multi-head causal attention (B=2, S=2048, D=1024, H=16) on 8 TRN2 NeuronCores.

Sharding: batch*head parallel. Core c handles batch b = c//4 and the 4
heads h in [4*(c%4), 4*(c%4)+4). Each core computes its heads' Q/K/V
projections (column-parallel), causal softmax attention, and its partial
row-parallel output projection; the host sums the 4 partial outputs per
batch (the AllReduce of row-parallel tensor parallelism).

v2 design vs v1 (311us baseline):
  - all matmul operands bf16 (host-cast): projection-phase DMA halves
    (24MB f32 -> 12MB bf16 of x per core), LDWEIGHTS cheaper.
  - xv host-shuffled chunk-major so every DMA line is 2KB contiguous.
  - psS/psO double-buffered: the attnV accumulation of q-block j+1 no
    longer waits for the softmax-normalize of block j (the 2.75us PE
    stall per block boundary that kept re-triggering the PE HAM
    half-clock throttle).
  - normalize chain shrunk to reciprocal([1,1024] on PSUM sums row) +
    gpsimd partition_broadcast + 2 DVE muls; PSUM->SBUF y copies moved
    to the idle GpSimd engine so Vector stays off the critical path.
  - causally-dead column windows of diagonal blocks are skipped in the
    score matmuls, exp, and attnV (cols < 128r of a diag-r tile are
    fully masked -> contribute exactly 0).
  - exp table preloaded via a dummy activation at t=0 (hides the ~2.7us
    ACT table-set load).
Softmax skips the max-subtraction: scores ~ N(0,1), so exp never
overflows fp32, and exp(-1e9/8) underflows to exactly 0 like the
reference's masked_fill(-1e9).
"""

import numpy as np

D_MODEL = 1024
N_HEADS = 16
D_K = 64
B, S = 2, 2048
N_CORES = 8
HPC = 4            # heads per core
KT = S // 128      # 16 k-tiles
QT = S // 512      # 4 q-tiles
ET = D_MODEL // 128  # 8 e-tiles (contraction tiles for projections)

WARM_MMS = 56
DIAG_TRIM = True   # skip fully-masked col windows of diagonal tiles
NEW_NORM = True    # recip+partition_broadcast normalize (vs v1-style)
SPREAD_OUTPROJ = True
V_SPLIT = True     # project V chunks 8-15 inside the pr0 attention pass

_PROG_CACHE = {}


def _build_program():
    import concourse.bacc as bacc_mod
    import concourse.mybir as mybir
    import concourse.tile as tile

    f32 = mybir.dt.float32
    bf16 = mybir.dt.bfloat16
    Exp = mybir.ActivationFunctionType.Exp

    nc = bacc_mod.Bacc(
        "TRN2", target_bir_lowering=False, debug=False, num_devices=N_CORES
    )

    xq = nc.dram_tensor("xq", [D_MODEL, S], bf16, kind="ExternalInput").ap()
    xk = nc.dram_tensor("xk", [D_MODEL, S], bf16, kind="ExternalInput").ap()
    xvc = nc.dram_tensor("xvc", [128, KT * ET * 128], bf16, kind="ExternalInput").ap()
    wq = nc.dram_tensor("wq", [128, ET * 256], bf16, kind="ExternalInput").ap()
    wk = nc.dram_tensor("wk", [128, ET * 256], bf16, kind="ExternalInput").ap()
    wv = nc.dram_tensor("wv", [128, ET * 256], bf16, kind="ExternalInput").ap()
    wo = nc.dram_tensor("wo", [256, D_MODEL], bf16, kind="ExternalInput").ap()
    maskt = nc.dram_tensor("maskt", [128, 2048], bf16, kind="ExternalInput").ap()
    idbf = nc.dram_tensor("idbf", [128, 196], bf16, kind="ExternalInput").ap()
    y = nc.dram_tensor("y", [S, D_MODEL], bf16, kind="ExternalOutput").ap()

    with (
        tile.TileContext(nc) as tc,
        nc.allow_low_precision("bf16 attention"),
        tc.tile_pool(name="persist", bufs=1) as pp,
    ):
        # ---- persistent SBUF tiles ----
        def persist(shape, dtype, name):
            return pp.tile(shape, dtype, name=name, tag=name)

        wq_sb = persist([128, ET * 256], bf16, "wq_sb")
        wk_sb = persist([128, ET * 256], bf16, "wk_sb")
        wv_sb = persist([128, ET * 256], bf16, "wv_sb")
        wo_sb = [persist([128, D_MODEL], bf16, f"wo_sb{p}") for p in range(2)]
        maskt_sb = persist([128, 2048], bf16, "maskt_sb")
        idbf_sb = persist([128, 196], bf16, "idbf_sb")
        qt_sb = [persist([128, S], bf16, f"qt_sb{p}") for p in range(2)]
        kt_sb = [persist([128, S], bf16, f"kt_sb{p}") for p in range(2)]
        v_sb = [persist([128, 260], bf16, f"v_sb{i}") for i in range(KT)]
        outt_sb = [persist([128, S], bf16, f"outt_sb{p}") for p in range(2)]
        exp_warm = persist([128, 1], f32, "exp_warm")

        nc.sync.dma_start(out=idbf_sb[:], in_=idbf[:])
        maskt_dram = maskt
        # preload the Exp table-set during the PE warm-up (~2.7us ACT load)
        nc.scalar.activation(exp_warm[:], idbf_sb[:, 0:1], Exp, scale=0.125)
        # ones columns of v_sb (col 64 of each 65-wide head slot) never
        # change: write them once, early, on gpsimd.
        for i in range(KT):
            nc.gpsimd.tensor_copy(
                v_sb[i][:].rearrange("p (h c) -> p h c", c=65)[:, :, 64:65],
                idbf_sb[:, 192:196].rearrange("p (h c) -> p h c", c=1),
            )
        nc.sync.dma_start(out=wq_sb[:], in_=wq[:])

        # ---- PE warm-up ----
        # The PE HAM clock gate starts at K=4/8 half-clock and returns to
        # full clock only after ~3.4us of gapless PE activity. Dense dummy
        # matmuls (results never read) force the transition while the
        # first x DMAs are in flight.
        with tc.tile_pool(name="psW", bufs=1, space="PSUM") as psW:
            wt = psW.tile([128, 128], f32, name="warm_start", tag="warm")
            for w in range(WARM_MMS):
                nc.tensor.matmul(
                    wt[:],
                    idbf_sb[:, 0:128],
                    idbf_sb[:, 64:192],
                    start=True,
                    stop=True,
                )

        # ---- phase B: projections ----
        # Q^T/K^T accumulate over all 8 e-tiles into [128, 2048] PSUM.
        # xv chunk DMAs (contiguous thanks to the host shuffle) are
        # interleaved so the V projection starts with its data resident.
        xvk_ctx = tc.tile_pool(name="xvk", bufs=16)
        xvkp = xvk_ctx.__enter__()
        with tc.tile_pool(name="xe", bufs=5) as xep:
            vdma_tiles = []

            def emit_v_dma():
                i = len(vdma_tiles)
                xvk = xvkp.tile([128, ET * 128], bf16, name=f"xvk_{i}", tag="xvk")
                nc.sync.dma_start(
                    out=xvk[:], in_=xvc[:, i * ET * 128 : (i + 1) * ET * 128]
                )
                vdma_tiles.append(xvk)

            psA_ctx = tc.tile_pool(name="psA", bufs=1, space="PSUM")
            psA = psA_ctx.__enter__()
            for ti, (x_dram, w_tile, dst) in enumerate(
                ((xq, wq_sb, qt_sb), (xk, wk_sb, kt_sb))
            ):
                ps = [
                    psA.tile(
                        [128, S], f32, name=f"ps_p{ti}_{m}", tag=f"proj{m}", bufs=1
                    )
                    for m in range(2)
                ]
                for e in range(ET):
                    xe = xep.tile([128, S], bf16, name=f"xe_{ti}_{e}", tag="xe")
                    nc.sync.dma_start(out=xe[:], in_=x_dram[e * 128 : (e + 1) * 128, :])
                    if ti == 0 and e == 1:
                        # prefetch mask + wk behind the first q x-tiles
                        nc.sync.dma_start(out=maskt_sb[:], in_=maskt_dram[:])
                        nc.sync.dma_start(out=wk_sb[:], in_=wk[:])
                    if ti == 1 and e == 0:
                        nc.sync.dma_start(out=wv_sb[:], in_=wv[:])
                        for p in range(2):
                            nc.sync.dma_start(
                                out=wo_sb[p][:], in_=wo[p * 128 : (p + 1) * 128, :]
                            )
                    if ti == 1 or e >= 1:
                        emit_v_dma()
                    for m in range(2):
                        lhsT = w_tile[:, e * 256 + m * 128 : e * 256 + (m + 1) * 128]
                        for n in range(QT):
                            nc.tensor.matmul(
                                ps[m][:, n * 512 : (n + 1) * 512],
                                lhsT,
                                xe[:, n * 512 : (n + 1) * 512],
                                start=(e == 0),
                                stop=(e == ET - 1),
                            )
                # PSUM -> SBUF casts: q on vector, k on scalar (parallel;
                # gpsimd cannot access PSUM on TRN2)
                for m in range(2):
                    if ti == 0:
                        nc.vector.tensor_copy(dst[m][:], ps[m][:])
                    else:
                        nc.scalar.activation(
                            dst[m][:], ps[m][:],
                            mybir.ActivationFunctionType.Copy,
                        )

            psA_ctx.__exit__(None, None, None)
            psV_ctx = tc.tile_pool(name="psV", bufs=2, space="PSUM")
            psV = psV_ctx.__enter__()
            # V projection: dense PE burst, v_sb tiles ready incrementally
            # (with V_SPLIT, chunks 8-15 are projected later, inside the
            # Act-bound pr0 attention blocks that first consume them)
            for i in range(KT // 2 if V_SPLIT else KT):
                if i >= len(vdma_tiles) - 2 and len(vdma_tiles) < KT:
                    emit_v_dma()
                psv = psV.tile([128, 256], f32, name=f"psv_{i}", tag="v")
                xvk = vdma_tiles[i]
                for e in range(ET):
                    nc.tensor.matmul(
                        psv[:],
                        xvk[:, e * 128 : (e + 1) * 128],
                        wv_sb[:, e * 256 : (e + 1) * 256],
                        start=(e == 0),
                        stop=(e == ET - 1),
                    )
                nc.vector.tensor_copy(
                    v_sb[i][:].rearrange("p (h c) -> p h c", c=65)[:, :, 0:64],
                    psv[:].rearrange("p (h d) -> p h d", d=64),
                )
            while len(vdma_tiles) < KT:
                emit_v_dma()
            psV_ctx.__exit__(None, None, None)

        # ---- phase C+D: attention with interleaved output projection ----
        # One head-pair per pass (pr = 0, 1), q-blocks ascending. Per
        # (pr, j): score tiles are [128, 1024] head-pair PSUM tiles; ONE
        # exp per round at FD~1024. attnV accumulates into a [65, 1024]
        # pair tile (row 64 = sum of exp via the ones column of v_sb).
        # Normalization of the previous block is emitted at the top of
        # the next block (psO is double-buffered so nothing on the PE
        # waits for it); output projection of block j-1 is spread over
        # i = 2..5 of block j during the pr=1 pass. Diag-r tiles skip
        # their fully-masked first 128*r columns everywhere.
        with (
            tc.tile_pool(name="psS", bufs=2, space="PSUM") as psS,
            tc.tile_pool(name="psO", bufs=2, space="PSUM") as psO,
            tc.tile_pool(name="et", bufs=6) as etp,
            tc.tile_pool(name="bcsb", bufs=3) as bcp,
            tc.tile_pool(name="rcsb", bufs=3) as rcp,
            tc.tile_pool(name="ysb", bufs=4) as ysbp,
        ):
            def emit_outproj_mtile(m):
                psy = psS.tile([128, 1024], f32, name=f"psy_{m}", tag="s")
                for p in range(2):
                    for n in range(2):
                        nc.tensor.matmul(
                            psy[:, n * 512 : (n + 1) * 512],
                            outt_sb[p][:, m * 128 : (m + 1) * 128],
                            wo_sb[p][:, n * 512 : (n + 1) * 512],
                            start=(p == 0),
                            stop=(p == 1),
                        )
                y_sb = ysbp.tile([128, 1024], bf16, name=f"y_sb_{m}", tag="ysb")
                nc.vector.tensor_copy(y_sb[:], psy[:])
                nc.sync.dma_start(out=y[m * 128 : (m + 1) * 128, :], in_=y_sb[:])

            def emit_normalize(pr, jj, ps_out_prev):
                qsj = slice(jj * 512, (jj + 1) * 512)
                if NEW_NORM:
                    # custom-DVE ops silently misread PSUM: stage the sums
                    # row into SBUF (partition 0) before the reciprocal.
                    # Split per head so the first multiply starts after a
                    # half-size copy+recip+broadcast chain.
                    bc_sb = bcp.tile(
                        [64, 1024], f32, name=f"bc_sb_{pr}_{jj}", tag="bc"
                    )
                    for hh in range(2):
                        hs = slice(512 * hh, 512 * (hh + 1))
                        ssb = rcp.tile(
                            [1, 512], f32, name=f"ssb_{pr}_{jj}_{hh}", tag="rc32"
                        )
                        nc.vector.tensor_copy(ssb[:], ps_out_prev[64:65, hs])
                        rc32 = rcp.tile(
                            [1, 512], f32, name=f"rc32_{pr}_{jj}_{hh}", tag="rc32"
                        )
                        nc.vector.reciprocal_approx_fast(out=rc32[:], in_=ssb[:])
                        nc.gpsimd.partition_broadcast(bc_sb[:, hs], rc32[:])
                else:
                    ssb = rcp.tile([33, 512], f32, name=f"ssb_{pr}_{jj}", tag="rc32")
                    for hh in range(2):
                        nc.vector.tensor_copy(
                            ssb[32 * hh : 32 * hh + 1, :],
                            ps_out_prev[64:65, 512 * hh : 512 * (hh + 1)],
                        )
                    rc32 = rcp.tile([33, 512], f32, name=f"rc_{pr}_{jj}", tag="rc32")
                    nc.vector.reciprocal_approx_fast(out=rc32[:], in_=ssb[:])
                    rcb = rcp.tile([33, 512], bf16, name=f"rcb_{pr}_{jj}", tag="rc32")
                    nc.vector.tensor_copy(rcb[:], rc32[:])
                    bcps = psS.tile([128, 1024], f32, name=f"bcp_{pr}_{jj}", tag="s")
                    for hh in range(2):
                        nc.tensor.matmul(
                            bcps[0:64, 512 * hh : 512 * (hh + 1)],
                            idbf_sb[32 * hh : 32 * hh + 1, 128:192],
                            rcb[32 * hh : 32 * hh + 1, :],
                            start=True,
                            stop=True,
                            tile_position=(32 * hh, 0),
                        )
                    bc_sb = bcp.tile(
                        [64, 1024], f32, name=f"bc_sb_{pr}_{jj}", tag="bc"
                    )
                    nc.vector.tensor_copy(bc_sb[:], bcps[0:64, :])
                for hh in range(2):
                    nc.vector.tensor_mul(
                        outt_sb[pr][64 * hh : 64 * hh + 64, qsj],
                        ps_out_prev[0:64, 512 * hh : 512 * (hh + 1)],
                        bc_sb[:, 512 * hh : 512 * (hh + 1)],
                    )

            pending_norm = None  # (pr, j, ps_out) awaiting lazy normalize
            pending_out = []     # outproj m-tiles awaiting emission (pr=1)
            carry = None         # final attnV of the previous block, emitted
                                 # after the next block's first scores+exp

            def emit_carry():
                c_et, c_i, c_w0, c_psout, c_pr, c_ni = carry
                for hh in range(2):
                    nc.tensor.matmul(
                        c_psout[:, 512 * hh + c_w0 : 512 * (hh + 1)],
                        v_sb[c_ni - 1][
                            :, (2 * c_pr + hh) * 65 : (2 * c_pr + hh + 1) * 65
                        ],
                        c_et[:, 512 * hh + c_w0 : 512 * (hh + 1)],
                        start=(c_ni - 1 == 0),
                        stop=True,
                    )

            for pr in range(2):
                for j in range(QT):
                    n_i = 4 * j + 4
                    ps_out = psO.tile(
                        [65, 1024], f32, name=f"ps_out_{pr}_{j}", tag="o"
                    )
                    prev_et = None
                    prev_i = -1
                    prev_w0 = 0
                    for i in range(n_i):
                        diag = i >= 4 * j
                        r = i - 4 * j
                        w0 = 128 * r if (diag and DIAG_TRIM) else 0
                        pss = psS.tile(
                            [128, 1024], f32, name=f"ps_s{pr}_{j}_{i}", tag="s"
                        )
                        for hh in range(2):
                            hp = slice(64 * hh, 64 * hh + 64)
                            nc.tensor.matmul(
                                pss[:, 512 * hh + w0 : 512 * (hh + 1)],
                                kt_sb[pr][hp, i * 128 : (i + 1) * 128],
                                qt_sb[pr][hp, j * 512 + w0 : (j + 1) * 512],
                                start=True,
                                stop=not diag,
                            )
                        if diag:
                            # band-only causal mask add (the straddle is
                            # 128 cols; beyond it the mask is zero)
                            bw = min(128 * (r + 1), 512)
                            for hh in range(2):
                                nc.tensor.matmul(
                                    pss[:, 512 * hh + w0 : 512 * hh + bw],
                                    idbf_sb[:, 0:128],
                                    maskt_sb[:, r * 512 + w0 : r * 512 + bw],
                                    start=False,
                                    stop=True,
                                )
                        et = etp.tile(
                            [128, 1024], bf16, name=f"et{pr}_{j}_{i}", tag="et"
                        )
                        if w0:
                            nc.scalar.activation(
                                et[:].rearrange("p (h q) -> p h q", h=2)[
                                    :, :, w0:512
                                ],
                                pss[:].rearrange("p (h q) -> p h q", h=2)[
                                    :, :, w0:512
                                ],
                                Exp,
                                scale=0.125,
                            )
                        else:
                            nc.scalar.activation(et[:], pss[:], Exp, scale=0.125)
                        if i == 0:
                            # previous block's final attnV, pipelined behind
                            # this block's first scores so its exp isn't
                            # delayed by the accumulation tail
                            if carry is not None:
                                emit_carry()
                                pending_norm = (carry[4], None, carry[3])
                                carry = None
                        else:
                            for hh in range(2):
                                nc.tensor.matmul(
                                    ps_out[:, 512 * hh + prev_w0 : 512 * (hh + 1)],
                                    v_sb[prev_i][
                                        :, (2 * pr + hh) * 65 : (2 * pr + hh + 1) * 65
                                    ],
                                    prev_et[:, 512 * hh + prev_w0 : 512 * (hh + 1)],
                                    start=(prev_i == 0),
                                    stop=False,
                                )
                        if i == 1 and pending_norm is not None:
                            pn_pr, pn_j, pn_psout = pending_norm
                            pn_j2 = (j - 1) % QT if pn_j is None else pn_j
                            emit_normalize(pn_pr, pn_j2, pn_psout)
                            pending_norm = None
                            if pn_pr == 1:
                                pending_out = list(
                                    range(4 * pn_j2, 4 * pn_j2 + 4)
                                )
                        if V_SPLIT and pr == 0 and j >= 2 and i < 4:
                            c = 4 * j + i
                            if i % 2 == 0:
                                vps = psS.tile(
                                    [128, 512], f32, name=f"psv_{c}", tag="s"
                                )
                            vc = slice(256 * (i % 2), 256 * (i % 2) + 256)
                            xvk = vdma_tiles[c]
                            for e in range(ET):
                                nc.tensor.matmul(
                                    vps[:, vc],
                                    xvk[:, e * 128 : (e + 1) * 128],
                                    wv_sb[:, e * 256 : (e + 1) * 256],
                                    start=(e == 0),
                                    stop=(e == ET - 1),
                                )
                            nc.vector.tensor_copy(
                                v_sb[c][:].rearrange("p (h c) -> p h c", c=65)[
                                    :, :, 0:64
                                ],
                                vps[:, vc].rearrange("p (h d) -> p h d", d=64),
                            )
                        prev_et, prev_i, prev_w0 = et, i, w0
                        if pending_out and i >= 2 and i % 2 == 0:
                            emit_outproj_mtile(pending_out.pop(0))
                    while pending_out:
                        emit_outproj_mtile(pending_out.pop(0))
                    carry = (prev_et, prev_i, prev_w0, ps_out, pr, n_i)
            # tail: final attnV + normalize + project the last q-block.
            # Dummy matmuls keep the PE HAM at full clock through the
            # normalize chain so the final outproj isn't half-clocked.
            emit_carry()
            emit_normalize(1, QT - 1, carry[3])
            wt = psS.tile([128, 512], f32, name="warm_tail", tag="s")
            for _ in range(16):
                nc.tensor.matmul(
                    wt[:], idbf_sb[:, 0:128], maskt_sb[:, 0:512],
                    start=True, stop=True,
                )
            for m in range(4 * (QT - 1), 4 * QT):
                emit_outproj_mtile(m)

        xvk_ctx.__exit__(None, None, None)

    nc.compile()
    return nc


def _get_program():
    if "nc" not in _PROG_CACHE:
        _PROG_CACHE["nc"] = _build_program()
    return _PROG_CACHE["nc"]


def _host_prep(query, key, value, mask, w_q, w_k, w_v, w_o):
    import ml_dtypes

    bf = ml_dtypes.bfloat16
    query = np.asarray(query, dtype=np.float32)
    key = np.asarray(key, dtype=np.float32)
    value = np.asarray(value, dtype=np.float32)
    w_q = np.asarray(w_q, dtype=np.float32)
    w_k = np.asarray(w_k, dtype=np.float32)
    w_v = np.asarray(w_v, dtype=np.float32)
    w_o = np.asarray(w_o, dtype=np.float32)
    m = np.asarray(mask).reshape(S, S).astype(bool)

    # The kernel's block-skip structure assumes the standard causal mask.
    expected = np.triu(np.ones((S, S), dtype=bool), k=1)
    if not np.array_equal(m, expected):
        raise NotImplementedError("kernel specialized for causal (triu, k=1) mask")

    # 4 canonical diagonal-straddle mask tiles: pattern r covers k-tile
    # 4j+r vs q-tile j; masked where (128r + row) > col.
    maskt = np.zeros((128, 2048), dtype=np.float32)
    rows = np.arange(128)[:, None]
    cols = np.arange(512)[None, :]
    for r in range(4):
        maskt[:, r * 512 : (r + 1) * 512] = np.where(
            (128 * r + rows) > cols, np.float32(-1e9), np.float32(0.0)
        )
    maskt = maskt.astype(bf)
    idbf = np.zeros((128, 196), dtype=bf)
    idbf[:, 0:128] = np.eye(128, dtype=bf)
    idbf[:, 128:196] = bf(1.0)

    def wslab(w, rs):
        # [1024 in, 256 out] -> [128, 8*256] with e-tile t at cols 256t
        wt = np.ascontiguousarray(w[rs, :].T).astype(bf)  # [1024, 256]
        return np.ascontiguousarray(
            wt.reshape(ET, 128, 256).transpose(1, 0, 2).reshape(128, ET * 256)
        )

    xt = {}
    for b in range(B):
        xt[("q", b)] = np.ascontiguousarray(query[b].T).astype(bf)
        xt[("k", b)] = np.ascontiguousarray(key[b].T).astype(bf)
        # chunk-major xv: chunk i contiguous [128, 1024] (2KB lines)
        vt = np.ascontiguousarray(value[b].T).astype(bf)
        xt[("v", b)] = np.ascontiguousarray(
            vt.reshape(ET, 128, KT, 128)
            .transpose(1, 2, 0, 3)
            .reshape(128, KT * ET * 128)
        )

    in_maps = []
    for c in range(N_CORES):
        b = c // 4
        hb = (c % 4) * HPC
        rs = slice(hb * D_K, (hb + HPC) * D_K)
        in_maps.append(
            {
                "xq": xt[("q", b)],
                "xk": xt[("k", b)],
                "xvc": xt[("v", b)],
                "wq": wslab(w_q, rs),
                "wk": wslab(w_k, rs),
                "wv": wslab(w_v, rs),
                "wo": np.ascontiguousarray(w_o[:, rs].T).astype(bf),
                "maskt": maskt,
                "idbf": idbf,
            }
        )
    return in_maps


def kernel(query, key, value, mask, w_q, w_k, w_v, w_o):
    from concourse.bass_utils import run_bass_kernel_spmd

    in_maps = _host_prep(query, key, value, mask, w_q, w_k, w_v, w_o)
    nc = _get_program()
    res = run_bass_kernel_spmd(nc, in_maps, list(range(N_CORES)))
    out = np.zeros((B, S, D_MODEL), dtype=np.float32)
    for c in range(N_CORES):
        out[c // 4] += res.results[c]["y"].astype(np.float32)
    return out
